# revision 6
# baseline (speedup 1.0000x reference)
"""Trainium2 Bass kernel for nn_GCNPairTwoConv (GNN message passing, 8 cores).

Sharding: the 256 graphs are split into 8 contiguous groups (same split for
both sides, balanced by total node count).  Each core owns its shard's conv
outputs and the edges whose dst falls in the shard.

Per conv layer, aggregation avoids per-edge scatters: the host sorts each
core's edges by 128-node dst window; a dma_gather fetches each edge's
source-feature row from a pair-packed bf16 table (pair indices fit int16);
a one-hot selection matrix (iota vs dst-rel compare on DVE) folds the
scatter and the pair-selection into PE matmuls accumulated in PSUM.
D^-1/2 source normalization is pre-scaled into table rows; dst-side scaling,
self-loop, bias, relu form a small per-window epilogue.

Inter-layer feature exchange runs host-side between three SPMD launches:
 A: table0 + attention-pool + conv0    B: conv1    C: conv2 + pools + MLP.
"""

import numpy as np

import jax
import concourse.bacc as bacc
import concourse.bass as bass
import concourse.mybir as mybir
import concourse.tile as tile
from concourse.masks import make_identity

F32 = mybir.dt.float32
BF16 = mybir.dt.bfloat16
I16 = mybir.dt.int16
OP = mybir.AluOpType
AF = mybir.ActivationFunctionType

N = 50000
F_IN = 64
H = 96
G = 256
NC = 8
P = 128
LPAD = 6400                 # padded local nodes per core
W_LOC = LPAD // P           # 50 local windows
TPAD = LPAD * NC            # 51200 padded table rows
W_FULL = TPAD // P          # 400 global windows


# ---------------------------------------------------------------------------
# host-side preprocessing (index metadata only)
# ---------------------------------------------------------------------------

def _graph_split(batch_p, batch_d):
    gb_p = np.searchsorted(batch_p, np.arange(G + 1))
    gb_d = np.searchsorted(batch_d, np.arange(G + 1))
    tot = gb_p + gb_d                       # cumulative p+d nodes per graph
    targets = (np.arange(1, NC) * tot[-1]) // NC
    gsplit = np.searchsorted(tot, targets)
    g0 = np.concatenate([[0], gsplit, [G]]).astype(np.int64)
    return g0, gb_p, gb_d


def _prep_side(edge_index, batch, g0, gb):
    src = np.asarray(edge_index[0])
    dst = np.asarray(edge_index[1])
    n0 = gb[g0]                             # node boundaries per core
    ln = np.diff(n0)
    assert ln.max() <= LPAD, ln.max()

    deg = np.bincount(dst, minlength=N).astype(np.float32)
    owner = np.searchsorted(n0, np.arange(N), side="right") - 1
    padrow = LPAD * owner + (np.arange(N) - n0[owner])

    eowner = np.searchsorted(n0, dst, side="right") - 1
    raw = []
    max_win = 0
    for k in range(NC):
        m = eowner == k
        s_k, d_k = src[m], dst[m]
        dloc = d_k - n0[k]
        win = dloc // P
        order = np.argsort(win, kind="stable")
        s_k, dloc, win = s_k[order], dloc[order], win[order]
        wstart = np.searchsorted(win, np.arange(W_LOC + 1))
        max_win = max(max_win, int(np.diff(wstart).max()))
        raw.append((s_k, dloc, wstart))

    cpw = int(np.ceil(max_win / P))
    spw = cpw * P
    side = {"deg": deg, "padrow": padrow, "n0": n0, "cpw": cpw, "cores": []}
    for k in range(NC):
        s_k, dloc, wstart = raw[k]
        gidx = np.zeros((W_LOC, spw), np.int16)
        drel = np.full((W_LOC, cpw, 2, P), -1.0, np.float32)
        for w in range(W_LOC):
            a, b = wstart[w], wstart[w + 1]
            cnt = b - a
            pr = padrow[s_k[a:b]]
            gidx[w, :cnt] = (pr >> 1).astype(np.int16)
            sub = (pr & 1).astype(np.int64)
            sl = np.arange(cnt)
            drel[w, sl // P, sub, sl % P] = dloc[a:b] - w * P
        wr = gidx.reshape(W_LOC, spw // 16, 16).transpose(0, 2, 1)
        side["cores"].append({
            "gidx": np.ascontiguousarray(np.tile(wr, (1, 8, 1))).astype(np.int16),
            "dstrel2": np.ascontiguousarray(
                drel.reshape(W_LOC, cpw * 2, P).transpose(2, 0, 1)
            ).reshape(P, W_LOC * cpw * 2).astype(np.float32),
            "lcnt": int(ln[k]), "nbase": int(n0[k]),
        })
    return side


def _winlay(v, nwin):
    return np.ascontiguousarray(np.asarray(v, np.float32).reshape(nwin, P).T)


def _host_prep(inputs):
    batch_p = np.asarray(inputs["x_p_batch"])
    batch_d = np.asarray(inputs["x_d_batch"])
    g0, gb_p, gb_d = _graph_split(batch_p, batch_d)
    pre = {"p": _prep_side(inputs["edge_index_p"], batch_p, g0, gb_p),
           "d": _prep_side(inputs["edge_index_d"], batch_d, g0, gb_d),
           "g0": g0}
    per_core = [dict() for _ in range(NC)]
    shared = {}
    for s, batch in (("p", batch_p), ("d", batch_d)):
        side = pre[s]
        x = np.asarray(inputs["x_p" if s == "p" else "x_d"], np.float32)
        xpad = np.zeros((TPAD, F_IN), np.float32)
        xpad[side["padrow"]] = x
        shared[f"xT_{s}"] = np.ascontiguousarray(xpad.T)
        degpad = np.zeros(TPAD, np.float32)
        degpad[side["padrow"]] = side["deg"]
        shared[f"degf_{s}"] = _winlay(degpad, W_FULL)
        cnt_all = np.bincount(batch, minlength=G).astype(np.float32)
        for k in range(NC):
            ck = side["cores"][k]
            nb, lc = ck["nbase"], ck["lcnt"]
            gb_, gc = g0[k], g0[k + 1] - g0[k]
            pc = per_core[k]
            pc[f"gidx_{s}"] = ck["gidx"]
            pc[f"dstrel_{s}"] = ck["dstrel2"]
            degl = np.zeros(LPAD, np.float32)
            degl[:lc] = side["deg"][nb:nb + lc]
            pc[f"degl_{s}"] = _winlay(degl, W_LOC)
            x65 = np.zeros((LPAD, F_IN + 1), np.float32)
            x65[:lc, :F_IN] = x[nb:nb + lc]
            x65[:lc, F_IN] = 1.0
            pc[f"x65_{s}"] = x65
            xTl = np.zeros((F_IN, LPAD), np.float32)
            xTl[:, :lc] = x[nb:nb + lc].T
            pc[f"xTl_{s}"] = xTl
            br = np.full(LPAD, -1.0, np.float32)
            br[:lc] = batch[nb:nb + lc] - gb_
            pc[f"batchrel_{s}"] = _winlay(br, W_LOC)
            cnt = np.ones((P, 1), np.float32)
            cnt[:gc, 0] = cnt_all[gb_:gb_ + gc]
            pc[f"cnt_{s}"] = cnt
    return pre, shared, per_core


# ---------------------------------------------------------------------------
# device program pieces
# ---------------------------------------------------------------------------

def _dinv_tile(nc, sb, deg_ap, nwin, name):
    """(deg+1)^-0.5 as [128, nwin] f32: add 1 -> reciprocal -> sqrt."""
    t = sb.tile([P, nwin], F32, name=name)
    nc.sync.dma_start(t[:], deg_ap[:])
    nc.vector.tensor_scalar_add(t[:], t[:], 1.0)
    nc.vector.reciprocal(t[:], t[:])
    nc.scalar.activation(t[:], t[:], AF.Sqrt)
    return t


def _iota(nc, sb, width, reps, name):
    t = sb.tile([P, reps * width], F32, name=name)
    nc.gpsimd.iota(t[:], pattern=[[0, reps], [1, width]], base=0,
                   channel_multiplier=0, allow_small_or_imprecise_dtypes=True)
    return t


def _ld(nc, sb, ap, shape, name, dt=F32):
    t = sb.tile(shape, dt, name=name)
    nc.sync.dma_start(t[:], ap[:])
    return t


def _conv_pass(nc, sb, ps, tag, cpw, pair_table, table_loc, gidx, dstrel,
               iota2, dinv_loc, w_next_ap, bias_ap, relu, t_next_loc,
               pool_sink=None):
    spw = cpw * P
    wn_t = None
    if w_next_ap is not None:
        wn_t = _ld(nc, sb, w_next_ap, [H, H], f"wn_{tag}")
    b_t = _ld(nc, sb, bias_ap, [P, H], f"b_{tag}")
    dr_t = _ld(nc, sb, dstrel, [P, W_LOC * cpw * 2], f"dr_{tag}")
    ident = sb.tile([P, P], F32, name=f"id_{tag}")
    make_identity(nc, ident[:])

    for w in range(W_LOC):
        idx_t = sb.tile([P, spw // 16], I16, tag="cidx", bufs=3)
        nc.sync.dma_start(idx_t[:], gidx[w])
        gbuf = sb.tile([P, cpw, 256], BF16, tag="gbuf", bufs=3)
        nc.gpsimd.dma_gather(
            out_ap=gbuf[:], in_ap=pair_table, idxs_ap=idx_t[:],
            num_idxs=spw, num_idxs_reg=spw, elem_size=256, single_packet=False)

        sbig = sb.tile([P, cpw * 2, P], BF16, tag="sbig", bufs=2)
        nc.vector.tensor_tensor(
            out=sbig[:], in0=iota2[:, :cpw * 2 * P]
                .rearrange("p (c v) -> p c v", v=P),
            in1=dr_t[:, w * cpw * 2:(w + 1) * cpw * 2]
                .rearrange("p (c o) -> p c o", o=1).to_broadcast([P, cpw * 2, P]),
            op=OP.is_equal)

        agg = ps.tile([P, H], F32, tag="agg", bufs=2, space="PSUM")
        nmm = cpw * 2
        for j in range(cpw):
            for s in (0, 1):
                c = 2 * j + s
                nc.tensor.matmul(
                    agg[:], lhsT=sbig[:, c, :],
                    rhs=gbuf[:, j, s * 128:s * 128 + H],
                    start=(c == 0), stop=(c == nmm - 1))

        # epilogue: h = relu( dinv*(agg + T_loc[w,:96]) + b )
        # (table rows are already dinv-scaled, so dinv*T_loc = dinv^2 * xw)
        tl = sb.tile([P, P], BF16, tag="tl", bufs=2)
        nc.sync.dma_start(tl[:], table_loc[w * P:(w + 1) * P, :])
        dcol = dinv_loc[:, w:w + 1]
        t1 = sb.tile([P, H], F32, tag="t1", bufs=2)
        nc.vector.tensor_copy(t1[:], tl[:, :H])
        h = sb.tile([P, H], F32, tag="h", bufs=2)
        nc.vector.tensor_add(h[:], t1[:], agg[:])
        nc.vector.tensor_scalar(h[:], h[:], dcol, None, op0=OP.mult)
        nc.vector.tensor_add(h[:], h[:], b_t[:])
        if relu:
            nc.vector.tensor_scalar_max(h[:], h[:], 0.0)

        if t_next_loc is not None:
            hT_ps = ps.tile([H, P], F32, tag="hT", bufs=2, space="PSUM")
            nc.tensor.transpose(out=hT_ps[:], in_=h[:], identity=ident[:])
            hT = sb.tile([H, P], F32, tag="hTs", bufs=2)
            nc.vector.tensor_copy(hT[:], hT_ps[:])
            tn_ps = ps.tile([P, H], F32, tag="tn", bufs=2, space="PSUM")
            nc.tensor.matmul(tn_ps[:], lhsT=hT[:], rhs=wn_t[:],
                             start=True, stop=True)
            tn = sb.tile([P, P], BF16, tag="tns", bufs=2)
            nc.gpsimd.memset(tn[:, H:], 0.0)
            nc.vector.tensor_scalar(tn[:, :H], tn_ps[:], dcol, None, op0=OP.mult)
            nc.sync.dma_start(t_next_loc[w * P:(w + 1) * P, :], tn[:])
        if pool_sink is not None:
            pool_ps, br_t, iota_g = pool_sink
            sp = sb.tile([P, P], F32, tag="sp", bufs=2)
            nc.vector.tensor_tensor(
                out=sp[:], in0=iota_g[:, :P],
                in1=br_t[:, w:w + 1].to_broadcast([P, P]), op=OP.is_equal)
            nc.tensor.matmul(pool_ps[:], lhsT=sp[:], rhs=h[:],
                             start=(w == 0), stop=(w == W_LOC - 1))


def _table_pass(nc, sb, ps, tag, xT_ap, dinv_t, w0_ap, kin, tdst, nwin):
    """T = dinv * (x @ W0) into tdst rows (bf16, 128-col padded)."""
    w0_t = _ld(nc, sb, w0_ap, [kin, H], f"w0_{tag}")
    BW = 8
    for wb in range(0, nwin, BW):
        nw = min(BW, nwin - wb)
        xt = sb.tile([kin, BW * P], F32, tag=f"xt_{tag[0]}", bufs=3)
        nc.sync.dma_start(xt[:, :nw * P], xT_ap[:, wb * P:(wb + nw) * P])
        for wi in range(nw):
            w = wb + wi
            t_ps = ps.tile([P, H], F32, tag="t0", bufs=2, space="PSUM")
            nc.tensor.matmul(t_ps[:], lhsT=xt[:, wi * P:(wi + 1) * P],
                             rhs=w0_t[:], start=True, stop=True)
            st = sb.tile([P, P], BF16, tag="t0s", bufs=3)
            nc.gpsimd.memset(st[:, H:], 0.0)
            nc.vector.tensor_scalar(st[:, :H], t_ps[:], dinv_t[:, w:w + 1],
                                    None, op0=OP.mult)
            nc.sync.dma_start(tdst[w * P:(w + 1) * P, :], st[:])


def _att_pass(nc, sb, ps, tag, xTl_ap, x65_ap, gw1_t, gb1_t, gw2_t, gb2_t,
              iota_g, br_t, att_out):
    pool_ps = ps.tile([P, F_IN + 1], F32, tag=f"attps_{tag}", space="PSUM")
    for w in range(W_LOC):
        xt = sb.tile([F_IN, P], F32, tag="xta", bufs=3)
        nc.sync.dma_start(xt[:], xTl_ap[:, w * P:(w + 1) * P])
        g1 = ps.tile([P, F_IN], F32, tag="g1", bufs=2, space="PSUM")
        nc.tensor.matmul(g1[:], lhsT=xt[:], rhs=gw1_t[:], start=True, stop=True)
        s1 = sb.tile([P, F_IN], F32, tag="s1", bufs=2)
        nc.vector.tensor_add(s1[:], g1[:], gb1_t[:])
        nc.vector.tensor_scalar_max(s1[:], s1[:], 0.0)
        prod = sb.tile([P, F_IN], F32, tag="prod", bufs=2)
        gate = sb.tile([P, 1], F32, tag="gate", bufs=2)
        nc.vector.tensor_tensor(out=prod[:], in0=s1[:], in1=gw2_t[:], op=OP.mult)
        nc.vector.tensor_reduce(out=gate[:], in_=prod[:],
                                axis=mybir.AxisListType.X, op=OP.add)
        nc.vector.tensor_add(gate[:], gate[:], gb2_t[:])
        e = sb.tile([P, 1], F32, tag="e", bufs=2)
        nc.scalar.activation(e[:], gate[:], AF.Exp)
        sg = sb.tile([P, P], F32, tag="sg", bufs=2)
        nc.vector.tensor_tensor(
            out=sg[:], in0=iota_g[:, :P],
            in1=br_t[:, w:w + 1].to_broadcast([P, P]), op=OP.is_equal)
        nc.vector.tensor_scalar(sg[:], sg[:], e[:], None, op0=OP.mult)
        xe = sb.tile([P, F_IN + 1], F32, tag="xe", bufs=3)
        nc.sync.dma_start(xe[:], x65_ap[w * P:(w + 1) * P, :])
        nc.tensor.matmul(pool_ps[:], lhsT=sg[:], rhs=xe[:],
                         start=(w == 0), stop=(w == W_LOC - 1))
    denom = sb.tile([P, 1], F32, name=f"den_{tag}")
    nc.vector.reciprocal(denom[:], pool_ps[:, F_IN:F_IN + 1])
    att = sb.tile([P, F_IN], F32, name=f"att_{tag}")
    nc.vector.tensor_scalar(att[:], pool_ps[:, :F_IN], denom[:], None,
                            op0=OP.mult)
    nc.sync.dma_start(att_out[:], att[:])


# ---------------------------------------------------------------------------
# the three launches
# ---------------------------------------------------------------------------

def _pairview(ap):
    return ap.rearrange("(q t) c -> q (t c)", t=2)


def _build_a(cpw_p, cpw_d):
    nc = bacc.Bacc("TRN2", target_bir_lowering=False, debug=False,
                   num_devices=NC)
    A = {}
    def inp(name, shape, dt=F32):
        A[name] = nc.dram_tensor(name, shape, dt, kind="ExternalInput").ap()
    for s, cpw in (("p", cpw_p), ("d", cpw_d)):
        inp(f"xT_{s}", [F_IN, TPAD])
        inp(f"degf_{s}", [P, W_FULL])
        inp(f"degl_{s}", [P, W_LOC])
        inp(f"xTl_{s}", [F_IN, LPAD])
        inp(f"x65_{s}", [LPAD, F_IN + 1])
        inp(f"batchrel_{s}", [P, W_LOC])
        inp(f"gidx_{s}", [W_LOC, P, cpw * P // 16], I16)
        inp(f"dstrel_{s}", [P, W_LOC * cpw * 2])
        inp(f"W0_{s}", [F_IN, H])
        inp(f"W1_{s}", [H, H])
        inp(f"b0_{s}", [P, H])
    for nm, sh in (("gW1", [F_IN, F_IN]), ("gb1", [P, F_IN]),
                   ("gW2r", [P, F_IN]), ("gb2r", [P, 1])):
        inp(nm, sh)
    outs = {}
    for s in ("p", "d"):
        outs[f"t1loc_{s}"] = nc.dram_tensor(
            f"t1loc_{s}", [LPAD, P], BF16, kind="ExternalOutput").ap()
        outs[f"att_{s}"] = nc.dram_tensor(
            f"att_{s}", [P, F_IN], F32, kind="ExternalOutput").ap()

    with tile.TileContext(nc) as tc:
        with tc.tile_pool(name="sb", bufs=1) as sb, \
             tc.tile_pool(name="dram", bufs=1, space="DRAM") as dp:
            iota_g = _iota(nc, sb, P, 1, "iota_g")
            iota2 = _iota(nc, sb, P, 2 * max(cpw_p, cpw_d), "iota2")
            gw1_t = _ld(nc, sb, A["gW1"], [F_IN, F_IN], "gw1")
            gb1_t = _ld(nc, sb, A["gb1"], [P, F_IN], "gb1t")
            gw2_t = _ld(nc, sb, A["gW2r"], [P, F_IN], "gw2t")
            gb2_t = _ld(nc, sb, A["gb2r"], [P, 1], "gb2t")

            t0_full, t0_loc, dinv_l = {}, {}, {}
            with tc.tile_pool(name="ps0", bufs=1, space="PSUM") as ps0:
                for s, cpw in (("p", cpw_p), ("d", cpw_d)):
                    dinv_f = _dinv_tile(nc, sb, A[f"degf_{s}"], W_FULL, f"df_{s}")
                    dinv_l[s] = _dinv_tile(nc, sb, A[f"degl_{s}"], W_LOC, f"dl_{s}")
                    t0 = dp.tile([TPAD, P], BF16, name=f"t0full_{s}")
                    _table_pass(nc, sb, ps0, f"f{s}", A[f"xT_{s}"], dinv_f,
                                A[f"W0_{s}"], F_IN, t0[:], W_FULL)
                    tl = dp.tile([LPAD, P], BF16, name=f"t0loc_{s}")
                    _table_pass(nc, sb, ps0, f"l{s}", A[f"xTl_{s}"], dinv_l[s],
                                A[f"W0_{s}"], F_IN, tl[:], W_LOC)
                    t0_full[s], t0_loc[s] = t0, tl

            with tc.tile_pool(name="ps1", bufs=1, space="PSUM") as ps1:
                for s in ("p", "d"):
                    br = _ld(nc, sb, A[f"batchrel_{s}"], [P, W_LOC], f"brA_{s}")
                    _att_pass(nc, sb, ps1, s, A[f"xTl_{s}"], A[f"x65_{s}"],
                              gw1_t, gb1_t, gw2_t, gb2_t, iota_g, br,
                              outs[f"att_{s}"])

            with tc.tile_pool(name="ps2", bufs=1, space="PSUM") as ps2:
                for s, cpw in (("p", cpw_p), ("d", cpw_d)):
                    _conv_pass(nc, sb, ps2, f"{s}0", cpw,
                               _pairview(t0_full[s][:]), t0_loc[s][:],
                               A[f"gidx_{s}"], A[f"dstrel_{s}"], iota2,
                               dinv_l[s], A[f"W1_{s}"], A[f"b0_{s}"],
                               relu=True, t_next_loc=outs[f"t1loc_{s}"])
    nc.compile()
    return nc


def _build_b(cpw_p, cpw_d):
    nc = bacc.Bacc("TRN2", target_bir_lowering=False, debug=False,
                   num_devices=NC)
    A = {}
    def inp(name, shape, dt=F32):
        A[name] = nc.dram_tensor(name, shape, dt, kind="ExternalInput").ap()
    for s, cpw in (("p", cpw_p), ("d", cpw_d)):
        inp(f"t1full_{s}", [TPAD, P], BF16)
        inp(f"t1loc_{s}", [LPAD, P], BF16)
        inp(f"degl_{s}", [P, W_LOC])
        inp(f"gidx_{s}", [W_LOC, P, cpw * P // 16], I16)
        inp(f"dstrel_{s}", [P, W_LOC * cpw * 2])
        inp(f"W2_{s}", [H, H])
        inp(f"b1_{s}", [P, H])
    outs = {s: nc.dram_tensor(f"t2loc_{s}", [LPAD, P], BF16,
                              kind="ExternalOutput").ap() for s in ("p", "d")}
    with tile.TileContext(nc) as tc:
        with tc.tile_pool(name="sb", bufs=1) as sb, \
             tc.tile_pool(name="ps", bufs=1, space="PSUM") as ps:
            iota2 = _iota(nc, sb, P, 2 * max(cpw_p, cpw_d), "iota2")
            for s, cpw in (("p", cpw_p), ("d", cpw_d)):
                dl = _dinv_tile(nc, sb, A[f"degl_{s}"], W_LOC, f"dl_{s}")
                _conv_pass(nc, sb, ps, f"{s}1", cpw,
                           _pairview(A[f"t1full_{s}"]), A[f"t1loc_{s}"],
                           A[f"gidx_{s}"], A[f"dstrel_{s}"], iota2, dl,
                           A[f"W2_{s}"], A[f"b1_{s}"], relu=True,
                           t_next_loc=outs[s])
    nc.compile()
    return nc


def _build_c(cpw_p, cpw_d):
    nc = bacc.Bacc("TRN2", target_bir_lowering=False, debug=False,
                   num_devices=NC)
    A = {}
    def inp(name, shape, dt=F32):
        A[name] = nc.dram_tensor(name, shape, dt, kind="ExternalInput").ap()
    for s, cpw in (("p", cpw_p), ("d", cpw_d)):
        inp(f"t2full_{s}", [TPAD, P], BF16)
        inp(f"t2loc_{s}", [LPAD, P], BF16)
        inp(f"degl_{s}", [P, W_LOC])
        inp(f"gidx_{s}", [W_LOC, P, cpw * P // 16], I16)
        inp(f"dstrel_{s}", [P, W_LOC * cpw * 2])
        inp(f"b2_{s}", [P, H])
        inp(f"batchrel_{s}", [P, W_LOC])
        inp(f"cnt_{s}", [P, 1])
        inp(f"att_{s}", [P, F_IN])
    for nm, sh in (("lW0", [3 * P, H]), ("lb0", [P, H]),
                   ("lW1r", [P, H]), ("lb1r", [P, 1])):
        inp(nm, sh)
    out = nc.dram_tensor("out_part", [P, 1], F32, kind="ExternalOutput").ap()

    with tile.TileContext(nc) as tc:
        with tc.tile_pool(name="sb", bufs=1) as sb, \
             tc.tile_pool(name="ps", bufs=1, space="PSUM") as ps:
            iota_g = _iota(nc, sb, P, 1, "iota_g")
            iota2 = _iota(nc, sb, P, 2 * max(cpw_p, cpw_d), "iota2")
            means = {}
            for s, cpw in (("p", cpw_p), ("d", cpw_d)):
                dl = _dinv_tile(nc, sb, A[f"degl_{s}"], W_LOC, f"dl_{s}")
                br = _ld(nc, sb, A[f"batchrel_{s}"], [P, W_LOC], f"br_{s}")
                pool_ps = ps.tile([P, H], F32, tag=f"pool_{s}", space="PSUM")
                _conv_pass(nc, sb, ps, f"{s}2", cpw,
                           _pairview(A[f"t2full_{s}"]), A[f"t2loc_{s}"],
                           A[f"gidx_{s}"], A[f"dstrel_{s}"], iota2, dl,
                           None, A[f"b2_{s}"], relu=False, t_next_loc=None,
                           pool_sink=(pool_ps, br, iota_g))
                cnt = _ld(nc, sb, A[f"cnt_{s}"], [P, 1], f"cnt_{s}")
                cinv = sb.tile([P, 1], F32, name=f"cinv_{s}")
                nc.vector.reciprocal(cinv[:], cnt[:])
                mean = sb.tile([P, H], F32, name=f"mean_{s}")
                nc.vector.tensor_scalar(mean[:], pool_ps[:], cinv[:], None,
                                        op0=OP.mult)
                means[s] = mean

            C = sb.tile([P, 2 * H + 2 * F_IN], F32, name="concat")
            nc.vector.tensor_copy(C[:, 0:H], means["p"][:])
            nc.vector.tensor_copy(C[:, H:2 * H], means["d"][:])
            attp = _ld(nc, sb, A["att_p"], [P, F_IN], "attp")
            attd = _ld(nc, sb, A["att_d"], [P, F_IN], "attd")
            nc.vector.tensor_copy(C[:, 2 * H:2 * H + F_IN], attp[:])
            nc.vector.tensor_copy(C[:, 2 * H + F_IN:], attd[:])

            ident = sb.tile([P, P], F32, name="id_c")
            make_identity(nc, ident[:])
            lw0 = [_ld(nc, sb, A["lW0"][c * P:(c + 1) * P, :], [P, H],
                       f"lw0_{c}") for c in range(3)]
            h1_ps = ps.tile([P, H], F32, tag="h1ps", space="PSUM")
            DIM = 2 * H + 2 * F_IN
            for c in range(3):
                wdt = min(P, DIM - c * P)
                ct_ps = ps.tile([P, P], F32, tag="ctps", bufs=2, space="PSUM")
                nc.tensor.transpose(out=ct_ps[:wdt, :],
                                    in_=C[:, c * P:c * P + wdt],
                                    identity=ident[:])
                ct = sb.tile([P, P], F32, tag="ct", bufs=2)
                nc.vector.tensor_copy(ct[:wdt, :], ct_ps[:wdt, :])
                nc.tensor.matmul(h1_ps[:], lhsT=ct[:wdt, :],
                                 rhs=lw0[c][:wdt, :],
                                 start=(c == 0), stop=(c == 2))
            lb0 = _ld(nc, sb, A["lb0"], [P, H], "lb0t")
            h1 = sb.tile([P, H], F32, name="h1")
            nc.vector.tensor_add(h1[:], h1_ps[:], lb0[:])
            nc.vector.tensor_scalar_max(h1[:], h1[:], 0.0)
            lw1 = _ld(nc, sb, A["lW1r"], [P, H], "lw1t")
            prod = sb.tile([P, H], F32, name="prodc")
            o = sb.tile([P, 1], F32, name="o")
            nc.vector.tensor_tensor(out=prod[:], in0=h1[:], in1=lw1[:], op=OP.mult)
            nc.vector.tensor_reduce(out=o[:], in_=prod[:],
                                    axis=mybir.AxisListType.X, op=OP.add)
            lb1 = _ld(nc, sb, A["lb1r"], [P, 1], "lb1t")
            nc.vector.tensor_add(o[:], o[:], lb1[:])
            nc.sync.dma_start(out[:], o[:])
    nc.compile()
    return nc


# ---------------------------------------------------------------------------
# persistent-jit SPMD execution (axon/PJRT path)
# ---------------------------------------------------------------------------

class _Runner:
    def __init__(self, nc, n_cores=NC):
        from concourse.bass2jax import (_bass_exec_p, install_neuronx_cc_hook,
                                        partition_id_tensor)
        from jax.sharding import Mesh, PartitionSpec
        from jax.experimental.shard_map import shard_map
        install_neuronx_cc_hook()
        self.n_cores = n_cores
        pname = nc.partition_id_tensor.name if nc.partition_id_tensor else None
        in_names, out_names, out_avals, zero_outs = [], [], [], []
        for alloc in nc.m.functions[0].allocations:
            if not isinstance(alloc, mybir.MemoryLocationSet):
                continue
            name = alloc.memorylocations[0].name
            if alloc.kind == "ExternalInput":
                if name != pname:
                    in_names.append(name)
            elif alloc.kind == "ExternalOutput":
                out_names.append(name)
                shape = tuple(alloc.tensor_shape)
                dtype = mybir.dt.np(alloc.dtype)
                out_avals.append(jax.core.ShapedArray(shape, dtype))
                zero_outs.append(np.zeros(shape, dtype))
        self.in_names, self.out_names = in_names, out_names
        self.n_params = len(in_names)
        all_in = list(in_names) + list(out_names)
        if pname is not None:
            all_in.append(pname)

        def _body(*args):
            operands = list(args)
            if pname is not None:
                operands.append(partition_id_tensor())
            return tuple(_bass_exec_p.bind(
                *operands, out_avals=tuple(out_avals), in_names=tuple(all_in),
                out_names=tuple(out_names), lowering_input_output_aliases=(),
                sim_require_finite=True, sim_require_nnan=True, nc=nc))

        devices = jax.devices()[:n_cores]
        self.mesh = Mesh(np.asarray(devices), ("core",))
        nin = self.n_params + len(out_names)
        self.fn = jax.jit(shard_map(
            _body, mesh=self.mesh, in_specs=(PartitionSpec("core"),) * nin,
            out_specs=(PartitionSpec("core"),) * len(out_names),
            check_rep=False), keep_unused=True)
        self.zero_outs = zero_outs

    def run(self, in_maps):
        per_core = [[np.ascontiguousarray(m[n]) for n in self.in_names]
                    for m in in_maps]
        args = [np.concatenate([per_core[c][i] for c in range(self.n_cores)], 0)
                for i in range(self.n_params)]
        args += [np.concatenate([z] * self.n_cores, 0) for z in self.zero_outs]
        out_arrs = self.fn(*args)
        res = []
        for c in range(self.n_cores):
            d = {}
            for i, name in enumerate(self.out_names):
                full = np.asarray(out_arrs[i])
                per = full.shape[0] // self.n_cores
                d[name] = full[c * per:(c + 1) * per]
            res.append(d)
        return res


_CACHE = {}


def _rep(v):
    return np.tile(np.asarray(v, np.float32).reshape(1, -1), (P, 1))


def kernel(**inputs):
    pre, shared, per_core = _host_prep(inputs)
    cpw_p, cpw_d = pre["p"]["cpw"], pre["d"]["cpw"]

    key = (cpw_p, cpw_d)
    if key not in _CACHE:
        nca, ncb, ncc = (_build_a(cpw_p, cpw_d), _build_b(cpw_p, cpw_d),
                         _build_c(cpw_p, cpw_d))
        _CACHE[key] = (_Runner(nca), _Runner(ncb), _Runner(ncc))
    ra, rb, rc = _CACHE[key]

    f32 = lambda t: np.asarray(inputs[t], np.float32)

    in_a = []
    for k in range(NC):
        m = {"gW1": f32("gW1"), "gb1": _rep(f32("gb1")),
             "gW2r": _rep(f32("gW2")[:, 0]),
             "gb2r": np.full((P, 1), float(f32("gb2")[0]), np.float32)}
        for s in ("p", "d"):
            pc = per_core[k]
            m[f"xT_{s}"] = shared[f"xT_{s}"]
            m[f"degf_{s}"] = shared[f"degf_{s}"]
            m[f"degl_{s}"] = pc[f"degl_{s}"]
            m[f"xTl_{s}"] = pc[f"xTl_{s}"]
            m[f"x65_{s}"] = pc[f"x65_{s}"]
            m[f"batchrel_{s}"] = pc[f"batchrel_{s}"]
            m[f"gidx_{s}"] = pc[f"gidx_{s}"]
            m[f"dstrel_{s}"] = pc[f"dstrel_{s}"]
            m[f"W0_{s}"] = f32(f"W{s}0")
            m[f"W1_{s}"] = f32(f"W{s}1")
            m[f"b0_{s}"] = _rep(f32(f"b{s}0"))
        in_a.append(m)
    res_a = ra.run(in_a)

    t1full = {s: np.concatenate([res_a[k][f"t1loc_{s}"] for k in range(NC)], 0)
              for s in ("p", "d")}

    in_b = []
    for k in range(NC):
        m = {}
        for s in ("p", "d"):
            pc = per_core[k]
            m[f"t1full_{s}"] = t1full[s]
            m[f"t1loc_{s}"] = res_a[k][f"t1loc_{s}"]
            m[f"degl_{s}"] = pc[f"degl_{s}"]
            m[f"gidx_{s}"] = pc[f"gidx_{s}"]
            m[f"dstrel_{s}"] = pc[f"dstrel_{s}"]
            m[f"W2_{s}"] = f32(f"W{s}2")
            m[f"b1_{s}"] = _rep(f32(f"b{s}1"))
        in_b.append(m)
    res_b = rb.run(in_b)

    t2full = {s: np.concatenate([res_b[k][f"t2loc_{s}"] for k in range(NC)], 0)
              for s in ("p", "d")}

    lW0 = f32("lW0")
    lW0p = np.zeros((3 * P, H), np.float32)
    lW0p[:lW0.shape[0]] = lW0
    in_c = []
    for k in range(NC):
        m = {"lW0": lW0p, "lb0": _rep(f32("lb0")),
             "lW1r": _rep(f32("lW1")[:, 0]),
             "lb1r": np.full((P, 1), float(f32("lb1")[0]), np.float32)}
        for s in ("p", "d"):
            pc = per_core[k]
            m[f"t2full_{s}"] = t2full[s]
            m[f"t2loc_{s}"] = res_b[k][f"t2loc_{s}"]
            m[f"degl_{s}"] = pc[f"degl_{s}"]
            m[f"gidx_{s}"] = pc[f"gidx_{s}"]
            m[f"dstrel_{s}"] = pc[f"dstrel_{s}"]
            m[f"b2_{s}"] = _rep(f32(f"b{s}2"))
            m[f"batchrel_{s}"] = pc[f"batchrel_{s}"]
            m[f"cnt_{s}"] = pc[f"cnt_{s}"]
            m[f"att_{s}"] = res_a[k][f"att_{s}"]
        in_c.append(m)
    res_c = rc.run(in_c)

    out = np.zeros((G, 1), np.float32)
    g0 = pre["g0"]
    for k in range(NC):
        gc = g0[k + 1] - g0[k]
        out[g0[k]:g0[k + 1], 0] = res_c[k]["out_part"][:gc, 0]
    return out
